# revision 1
# baseline (speedup 1.0000x reference)
"""GQA attention (B=2, S=1024, D=2048, 32 q heads / 8 kv heads, RoPE, causal)
on 8 TRN2 NeuronCores.

Strategy: pure data parallel — core c handles batch b = c // 4 and two
128-token blocks {j, 7-j} (j = c % 4) of that batch, which balances causal
attention work exactly (j+1 + 8-j = 9 kv-tiles per head for every core).
Each core computes full K/V for its batch (replicated within the 4-core
batch group), Q for its 256 tokens, attention, and its 256 rows of the
output projection. No collectives.

Layouts:
  - Q^T/K^T kept as [d, tok] (d on partitions) so scores^T[kt, qt] comes out
    of the PE directly with softmax's reduction (over kt) computable by
    matmul against a ones column appended to V.
  - RoPE done in deinterleaved space: Wq/Wk columns are permuted on the host
    (per-head even dims then odd dims), which leaves attention scores
    invariant; on device rope is t*C + swap(t)*D with host-built C/D tables
    and swap via a PE permutation matmul.
  - Causal mask applied multiplicatively (0/1, fractional on the diagonal
    tiles) to exp(scores) — mask content is per-core DATA so the SPMD
    instruction stream stays uniform.
"""

import numpy as np
import ml_dtypes

import concourse.bass as bass
import concourse.tile as tile
from concourse import bacc
from concourse import mybir
from concourse.bass_utils import run_bass_kernel_spmd

BF16 = ml_dtypes.bfloat16
D_MODEL = 2048
N_HEAD = 32
N_KV = 8
N_REP = 4
DK = 64
HALF = 32
THETA = 10000.0
B, S = 2, 1024
NT = S // 128  # 8 kv tiles of 128
QT = 256  # q tokens per core (two blocks of 128)

_cache = {}


def _build_nc(phases=3):
    nc = bacc.Bacc("TRN2", target_bir_lowering=False, debug=False)
    f32 = mybir.dt.float32
    bf16 = mybir.dt.bfloat16

    # ---- DRAM parameters (per-core shards supplied via in_maps) ----
    xT = nc.declare_dram_parameter("xT", [D_MODEL, S], bf16, isOutput=False)
    xqT = nc.declare_dram_parameter("xqT", [D_MODEL, QT], bf16, isOutput=False)
    wq = nc.declare_dram_parameter("wq", [D_MODEL, D_MODEL], bf16, isOutput=False)
    wk = nc.declare_dram_parameter("wk", [D_MODEL, 512], bf16, isOutput=False)
    wv = nc.declare_dram_parameter("wv", [D_MODEL, 512], bf16, isOutput=False)
    wo = nc.declare_dram_parameter("wo", [D_MODEL, D_MODEL], bf16, isOutput=False)
    bqr = nc.declare_dram_parameter("bqr", [1, D_MODEL], bf16, isOutput=False)
    bkr = nc.declare_dram_parameter("bkr", [1, 512], bf16, isOutput=False)
    bvr = nc.declare_dram_parameter("bvr", [1, 512], bf16, isOutput=False)
    bor = nc.declare_dram_parameter("bor", [1, D_MODEL], bf16, isOutput=False)
    ckt = nc.declare_dram_parameter("ckt", [128, S], bf16, isOutput=False)
    dkt = nc.declare_dram_parameter("dkt", [128, S], bf16, isOutput=False)
    cqt = nc.declare_dram_parameter("cqt", [128, QT], bf16, isOutput=False)
    dqt = nc.declare_dram_parameter("dqt", [128, QT], bf16, isOutput=False)
    pswap = nc.declare_dram_parameter("pswap", [128, 128], bf16, isOutput=False)
    packa = nc.declare_dram_parameter("packa", [64, 128], bf16, isOutput=False)
    packb = nc.declare_dram_parameter("packb", [64, 128], bf16, isOutput=False)
    onesb = nc.declare_dram_parameter("onesb", [65, 64], f32, isOutput=False)
    # mask[kt_local, i*256 + blk*128 + q_local] in {0, 1} (fractional = causal tri)
    maskT = nc.declare_dram_parameter("maskT", [128, NT * QT], bf16, isOutput=False)
    out = nc.declare_dram_parameter("out", [QT, D_MODEL], f32, isOutput=True)

    with tile.TileContext(nc) as tc:
        import contextlib

        with contextlib.ExitStack() as es:
            singles = es.enter_context(tc.tile_pool(name="singles", bufs=1))
            work = es.enter_context(tc.tile_pool(name="work", bufs=4))
            psA = es.enter_context(tc.tile_pool(name="psA", bufs=2, space="PSUM"))
            psB = es.enter_context(tc.tile_pool(name="psB", bufs=4, space="PSUM"))

            # ---- persistent constants / tables ----
            ck_sb = singles.tile([128, S], bf16)
            dk_sb = singles.tile([128, S], bf16)
            cq_sb = singles.tile([128, QT], bf16)
            dq_sb = singles.tile([128, QT], bf16)
            psw_sb = singles.tile([128, 128], bf16)
            pka_sb = singles.tile([64, 128], bf16)
            pkb_sb = singles.tile([64, 128], bf16)
            ones_sb = singles.tile([65, 64], f32)
            mask_sb = singles.tile([128, NT * QT], bf16)
            bq_sb = singles.tile([1, D_MODEL], bf16)
            bk_sb = singles.tile([1, 512], bf16)
            bv_sb = singles.tile([1, 512], bf16)
            bo_sb = singles.tile([1, D_MODEL], bf16)
            ones_row = singles.tile([1, 512], bf16)
            nc.vector.memset(ones_row, 1.0)
            for t, src in [
                (ck_sb, ckt), (dk_sb, dkt), (cq_sb, cqt), (dq_sb, dqt),
                (psw_sb, pswap), (pka_sb, packa), (pkb_sb, packb),
                (ones_sb, onesb), (mask_sb, maskT),
                (bq_sb, bqr), (bk_sb, bkr), (bv_sb, bvr), (bo_sb, bor),
            ]:
                nc.sync.dma_start(out=t, in_=src[:])
            # bq [2048] -> [128, 16]: col m holds bq[128m : 128m+128]

            # ---- persistent activations ----
            ropek = [singles.tile([64, S], bf16, name=f"ropek{i}", tag=f"ropek{i}") for i in range(N_KV)]
            ropeq = [singles.tile([64, QT], bf16, name=f"ropeq{i}", tag=f"ropeq{i}") for i in range(N_HEAD)]
            vp = [singles.tile([128, NT * 65], bf16, name=f"vp{i}", tag=f"vp{i}") for i in range(N_KV)]
            attT = [singles.tile([128, QT], bf16, name=f"attT{i}", tag=f"attT{i}") for i in range(N_HEAD // 2)]

            # =========== Phase 1: projections + rope ===========
            with contextlib.ExitStack() as proj_es:
                ppool = proj_es.enter_context(tc.tile_pool(name="proj", bufs=1))
                wqpool = proj_es.enter_context(tc.tile_pool(name="wqp", bufs=3))

                xT_sb = [ppool.tile([128, S], bf16, name=f"xt{kk}", tag=f"xt{kk}") for kk in range(16)]
                xq_sb = [ppool.tile([128, QT], bf16, name=f"xq{kk}", tag=f"xq{kk}") for kk in range(16)]
                wk_sb = [ppool.tile([128, 512], bf16, name=f"wk{kk}", tag=f"wk{kk}") for kk in range(16)]
                wv_sb = [ppool.tile([128, 512], bf16, name=f"wv{kk}", tag=f"wv{kk}") for kk in range(16)]
                for kk in range(16):
                    r = slice(kk * 128, kk * 128 + 128)
                    nc.sync.dma_start(out=xT_sb[kk], in_=xT[r, :])
                    nc.sync.dma_start(out=xq_sb[kk], in_=xqT[r, :])
                    nc.sync.dma_start(out=wk_sb[kk], in_=wk[r, :])
                    nc.sync.dma_start(out=wv_sb[kk], in_=wv[r, :])

                # ---- K^T = wk^T @ xT, rope -> ropek[64, S] per kv head ----
                for m in range(4):
                    kps = psA.tile([128, S], mybir.dt.float32, tag="A")
                    for hf in range(2):
                        cols = slice(hf * 512, hf * 512 + 512)
                        for kk in range(16):
                            nc.tensor.matmul(
                                kps[:, cols],
                                wk_sb[kk][:, m * 128:m * 128 + 128],
                                xT_sb[kk][:, cols],
                                start=(kk == 0), stop=False,
                            )
                        nc.tensor.matmul(
                            kps[:, cols],
                            bk_sb[:, m * 128:m * 128 + 128],
                            ones_row[:, 0:512],
                            start=False, stop=True,
                        )
                    k_sb = work.tile([128, S], bf16, tag="ksb")
                    nc.vector.tensor_copy(k_sb, kps)
                    swp = psA.tile([128, S], mybir.dt.float32, tag="A")
                    for hf in range(2):
                        cols = slice(hf * 512, hf * 512 + 512)
                        nc.tensor.matmul(swp[:, cols], psw_sb, k_sb[:, cols],
                                         start=True, stop=True)
                    t1 = work.tile([128, S], bf16, tag="t1")
                    t2 = work.tile([128, S], bf16, tag="t2")
                    nc.vector.tensor_mul(t1, k_sb, ck_sb)
                    nc.vector.tensor_mul(t2, swp, dk_sb)
                    nc.vector.tensor_add(ropek[2 * m], t1[0:64, :], t2[0:64, :])
                    nc.vector.tensor_add(ropek[2 * m + 1], t1[64:128, :], t2[64:128, :])

                # ---- V[t, dv] + bias -> vp tiles with ones column ----
                for h in range(N_KV):
                    nc.vector.memset(vp[h], 1.0)
                for i in range(NT):
                    vps = psA.tile([128, 512], mybir.dt.float32, tag="A")
                    for kk in range(16):
                        nc.tensor.matmul(
                            vps,
                            xT_sb[kk][:, i * 128:i * 128 + 128],
                            wv_sb[kk],
                            start=(kk == 0), stop=False,
                        )
                    nc.tensor.matmul(
                        vps,
                        ones_row[:, 0:128],
                        bv_sb,
                        start=False, stop=True,
                    )
                    for h in range(N_KV):
                        nc.vector.tensor_copy(
                            vp[h][:, i * 65:i * 65 + 64],
                            vps[:, h * 64:h * 64 + 64],
                        )

                # ---- Q^T = wq^T @ xqT, rope -> ropeq[64, QT] per head ----
                for m in range(16):
                    wqm = wqpool.tile([128, 16, 128], bf16, tag="wqm")
                    # wq[:, m*128 : m*128+128] laid out as [p, kk, c]
                    nc.sync.dma_start(
                        out=wqm,
                        in_=wq[:, m * 128:m * 128 + 128].rearrange(
                            "(kk p) c -> p kk c", p=128),
                    )
                    qps = psA.tile([128, QT], mybir.dt.float32, tag="A")
                    for kk in range(16):
                        nc.tensor.matmul(
                            qps,
                            wqm[:, kk, :],
                            xq_sb[kk],
                            start=(kk == 0), stop=False,
                        )
                    nc.tensor.matmul(
                        qps,
                        bq_sb[:, m * 128:m * 128 + 128],
                        ones_row[:, 0:QT],
                        start=False, stop=True,
                    )
                    q_sb = work.tile([128, QT], bf16, tag="qsb")
                    nc.vector.tensor_copy(q_sb, qps)
                    swq = psB.tile([128, QT], mybir.dt.float32, tag="B")
                    nc.tensor.matmul(swq, psw_sb, q_sb, start=True, stop=True)
                    t1 = work.tile([128, QT], bf16, tag="qt1")
                    t2 = work.tile([128, QT], bf16, tag="qt2")
                    nc.vector.tensor_mul(t1, q_sb, cq_sb)
                    nc.vector.tensor_mul(t2, swq, dq_sb)
                    nc.vector.tensor_add(ropeq[2 * m], t1[0:64, :], t2[0:64, :])
                    nc.vector.tensor_add(ropeq[2 * m + 1], t1[64:128, :], t2[64:128, :])

            # =========== Phase 2: attention ===========
            if phases < 2:
                return nc
            with contextlib.ExitStack() as att_es:
                apool = att_es.enter_context(tc.tile_pool(name="att", bufs=3))
                dpool = att_es.enter_context(tc.tile_pool(name="div", bufs=4))

                for pr in range(N_HEAD // 2):
                    attq = [None, None]
                    for sub in range(2):
                        h = 2 * pr + sub
                        kvh = h // N_REP
                        probs = apool.tile([128, NT * QT], bf16, tag="probs")
                        for half in range(2):
                            sps = psA.tile([128, 4 * QT], mybir.dt.float32, tag="A")
                            for ii in range(4):
                                i = half * 4 + ii
                                nc.tensor.matmul(
                                    sps[:, ii * QT:(ii + 1) * QT],
                                    ropek[kvh][:, i * 128:i * 128 + 128],
                                    ropeq[h],
                                    start=True, stop=True,
                                )
                            nc.scalar.activation(
                                probs[:, half * 4 * QT:(half + 1) * 4 * QT],
                                sps,
                                mybir.ActivationFunctionType.Exp,
                                bias=0.0, scale=0.125,
                            )
                        nc.vector.tensor_mul(probs, probs, mask_sb)
                        outv = psB.tile([65, QT], mybir.dt.float32, tag="B")
                        for i in range(NT):
                            nc.tensor.matmul(
                                outv,
                                vp[kvh][:, i * 65:i * 65 + 65],
                                probs[:, i * QT:(i + 1) * QT],
                                start=(i == 0), stop=(i == NT - 1),
                            )
                        rd = dpool.tile([65, QT], mybir.dt.float32, tag="rd")
                        nc.vector.reciprocal(rd[64:65, :], outv[64:65, :])
                        bcp = psB.tile([64, QT], mybir.dt.float32, tag="B")
                        nc.tensor.matmul(bcp, ones_sb[64:65, 0:64], rd[64:65, :],
                                         start=True, stop=True)
                        bc_sb = dpool.tile([64, QT], mybir.dt.float32, tag="bcs")
                        nc.vector.tensor_copy(bc_sb, bcp)
                        aq = dpool.tile([64, QT], bf16, tag=f"aq{sub}")
                        nc.vector.tensor_mul(aq, outv[0:64, :], bc_sb)
                        attq[sub] = aq
                    pk = psB.tile([128, QT], mybir.dt.float32, tag="B")
                    nc.tensor.matmul(pk, pka_sb, attq[0], start=True, stop=False)
                    nc.tensor.matmul(pk, pkb_sb, attq[1], start=False, stop=True)
                    nc.vector.tensor_copy(attT[pr], pk)

            # =========== Phase 3: output projection ===========
            if phases < 3:
                return nc
            with contextlib.ExitStack() as op_es:
                wopool = op_es.enter_context(tc.tile_pool(name="wop", bufs=2))
                opool = op_es.enter_context(tc.tile_pool(name="osb", bufs=3))
                for n in range(4):
                    won = wopool.tile([128, 16, 512], bf16, tag="won")
                    nc.sync.dma_start(
                        out=won,
                        in_=wo[:, n * 512:n * 512 + 512].rearrange(
                            "(p q) c -> q p c", q=128),
                    )
                    for blk in range(2):
                        ops = psA.tile([128, 512], mybir.dt.float32, tag="A")
                        for p in range(16):
                            nc.tensor.matmul(
                                ops,
                                attT[p][:, blk * 128:blk * 128 + 128],
                                won[:, p, :],
                                start=(p == 0), stop=False,
                            )
                        nc.tensor.matmul(
                            ops,
                            ones_row[:, 0:128],
                            bo_sb[:, n * 512:n * 512 + 512],
                            start=False, stop=True,
                        )
                        osb = opool.tile([128, 512], mybir.dt.float32, tag="osb")
                        nc.vector.tensor_copy(osb, ops)
                        nc.sync.dma_start(
                            out=out[blk * 128:blk * 128 + 128, n * 512:n * 512 + 512],
                            in_=osb,
                        )
    return nc


def _host_prep(x, Wq, bq, Wk, bk, Wv, bv, Wo, bo):
    """Build per-core input maps."""
    # per-head even/odd deinterleave permutation of output columns
    def colperm(nheads):
        p = []
        for h in range(nheads):
            base = h * DK
            p.extend([base + 2 * j for j in range(HALF)])
            p.extend([base + 2 * j + 1 for j in range(HALF)])
        return np.array(p)

    qperm = colperm(N_HEAD)
    kperm = colperm(N_KV)
    wq_p = np.ascontiguousarray(Wq[:, qperm]).astype(BF16)
    wk_p = np.ascontiguousarray(Wk[:, kperm]).astype(BF16)
    bq_p = np.ascontiguousarray(bq[qperm]).astype(BF16).reshape(1, D_MODEL)
    bk_p = np.ascontiguousarray(bk[kperm]).astype(BF16).reshape(1, 512)
    wv_c = Wv.astype(BF16)
    wo_c = Wo.astype(BF16)
    bv_r = bv.astype(BF16).reshape(1, 512)
    bo_r = bo.astype(BF16).reshape(1, D_MODEL)

    invf = THETA ** (-(np.arange(HALF, dtype=np.float64) * 2.0 / DK))
    posf = np.arange(S, dtype=np.float64)
    ang = posf[:, None] * invf[None, :]  # [S, 32]
    cos_t, sin_t = np.cos(ang), np.sin(ang)

    def rope_tables(pos_idx):
        # [128, len(pos_idx)] tables in deinterleaved space (2 heads / 128 rows)
        n = len(pos_idx)
        C = np.zeros((128, n), np.float32)
        D = np.zeros((128, n), np.float32)
        for p in range(128):
            r = p % DK
            i = r if r < HALF else r - HALF
            C[p] = cos_t[pos_idx, i]
            D[p] = (-sin_t[pos_idx, i]) if r < HALF else sin_t[pos_idx, i]
        return C.astype(BF16), D.astype(BF16)

    ckt, dkt = rope_tables(np.arange(S))

    psw = np.zeros((128, 128), np.float32)
    for m in range(128):
        k = m + HALF if (m % DK) < HALF else m - HALF
        psw[k, m] = 1.0
    psw = psw.astype(BF16)
    pka = np.zeros((64, 128), np.float32)
    pkb = np.zeros((64, 128), np.float32)
    for k in range(64):
        pka[k, k] = 1.0
        pkb[k, k + 64] = 1.0
    pka, pkb = pka.astype(BF16), pkb.astype(BF16)
    ones65 = np.ones((65, 64), np.float32)

    in_maps = []
    meta = []
    for c in range(8):
        b, j = c // 4, c % 4
        blocks = [j, 7 - j]
        qrows = np.concatenate([np.arange(bb * 128, bb * 128 + 128) for bb in blocks])
        xb = np.asarray(x[b], dtype=np.float32)
        xT = np.ascontiguousarray(xb.T).astype(BF16)
        xqT = np.ascontiguousarray(xb[qrows].T).astype(BF16)
        cqt, dqt = rope_tables(qrows)
        # mask[kt_local, i*QT + blk*128 + ql] = 1 if (i*128+kt_local) <= qpos else 0
        mask = np.zeros((128, NT * QT), np.float32)
        kt_local = np.arange(128)
        for i in range(NT):
            ktg = i * 128 + kt_local
            for blki, bb in enumerate(blocks):
                qpos = bb * 128 + np.arange(128)
                mask[:, i * QT + blki * 128:i * QT + blki * 128 + 128] = (
                    ktg[:, None] <= qpos[None, :]
                )
        in_maps.append({
            "xT": xT, "xqT": xqT, "wq": wq_p, "wk": wk_p, "wv": wv_c, "wo": wo_c,
            "bqr": bq_p, "bkr": bk_p, "bvr": bv_r, "bor": bo_r,
            "ckt": ckt, "dkt": dkt, "cqt": cqt, "dqt": dqt,
            "pswap": psw, "packa": pka, "packb": pkb, "onesb": ones65,
            "maskT": mask.astype(BF16),
        })
        meta.append((b, blocks))
    return in_maps, meta


def kernel(x, Wq, bq, Wk, bk, Wv, bv, Wo, bo):
    if "nc" not in _cache:
        nc0 = _build_nc()
        nc0.finalize()
        _cache["nc"] = nc0
    nc = _cache["nc"]
    in_maps, meta = _host_prep(x, Wq, bq, Wk, bk, Wv, bv, Wo, bo)
    res = run_bass_kernel_spmd(nc, in_maps, list(range(8)))
    full = np.zeros((B, S, D_MODEL), np.float32)
    for c in range(8):
        b, blocks = meta[c]
        o = res.results[c]["out"]
        for blki, bb in enumerate(blocks):
            full[b, bb * 128:bb * 128 + 128] = o[blki * 128:(blki + 1) * 128]
    return full



# revision 8
# speedup vs baseline: 1.2594x; 1.2594x over previous
"""GQA attention (B=2, S=1024, D=2048, 32 q heads / 8 kv heads, RoPE, causal)
on 8 TRN2 NeuronCores.

Strategy (v2): data parallel on batch (4 cores per batch), with the K/V
projections tensor-parallel *within* each batch group: core with group rank r
computes kv heads {2r, 2r+1} only (1/4 of the old replicated work), ropes K
locally, and the group AllGathers the K^T / V chunks through HBM while the
(long) Q projection runs.  Attention itself stays data-parallel: core r
handles q-token blocks {r, 7-r} of its batch (9 causal kv-tiles of work for
every core).  Causal structure is exploited uniformly: block A = r < 4 never
attends past kv tile 3, so score tiles 4-7 are computed for block B only
(25% fewer score columns + exp work); the multiplicative mask (per-core DATA)
zeroes everything invalid, so the SPMD instruction stream is identical on all
cores.

Softmax denominators are batched: each head's denominator row (from the ones
column appended to V) is copied to one partition of a [32, 256] tile via
SBUF->SBUF DMA, a single reciprocal + tiny broadcast matmuls replace 32
single-partition reciprocals.  Bias adds ride the scalar engine's PSUM->SBUF
copy (per-partition bias) for Q/K/V^T.  Wq/Wo are pre-swizzled on the host so
all weight DMAs are contiguous.
"""

import numpy as np
import ml_dtypes

import concourse.bass as bass
import concourse.tile as tile
from concourse import bacc
from concourse import mybir
from concourse.bass_utils import run_bass_kernel_spmd

BF16 = ml_dtypes.bfloat16
D_MODEL = 2048
N_HEAD = 32
N_KV = 8
N_REP = 4
DK = 64
HALF = 32
THETA = 10000.0
B, S = 2, 1024
NT = S // 128  # 8 kv tiles of 128
QT = 256  # q tokens per core (two blocks of 128)
GROUPS = [[0, 1, 2, 3], [4, 5, 6, 7]]

_cache = {}


def _build_nc():
    nc = bacc.Bacc("TRN2", target_bir_lowering=False, debug=False)
    f32 = mybir.dt.float32
    bf16 = mybir.dt.bfloat16

    # ---- DRAM parameters (per-core shards supplied via in_maps) ----
    xT = nc.declare_dram_parameter("xT", [128, 16, S], bf16, isOutput=False)
    xq = nc.declare_dram_parameter("xq", [128, 16, QT], bf16, isOutput=False)
    wq = nc.declare_dram_parameter("wq", [128, 16, 16, 128], bf16, isOutput=False)
    wk = nc.declare_dram_parameter("wk", [128, 16, 128], bf16, isOutput=False)
    wv = nc.declare_dram_parameter("wv", [128, 16, 128], bf16, isOutput=False)
    wo = nc.declare_dram_parameter("wo", [128, 4, 16, 512], bf16, isOutput=False)
    bq = nc.declare_dram_parameter("bq", [128, 16], f32, isOutput=False)
    bk = nc.declare_dram_parameter("bk", [128, 1], f32, isOutput=False)
    bv = nc.declare_dram_parameter("bv", [128, 1], f32, isOutput=False)
    bo = nc.declare_dram_parameter("bo", [1, D_MODEL], bf16, isOutput=False)
    ckt = nc.declare_dram_parameter("ckt", [128, S], bf16, isOutput=False)
    dkt = nc.declare_dram_parameter("dkt", [128, S], bf16, isOutput=False)
    cqt = nc.declare_dram_parameter("cqt", [128, QT], bf16, isOutput=False)
    dqt = nc.declare_dram_parameter("dqt", [128, QT], bf16, isOutput=False)
    pswap = nc.declare_dram_parameter("pswap", [128, 128], bf16, isOutput=False)
    packa = nc.declare_dram_parameter("packa", [64, 128], bf16, isOutput=False)
    packb = nc.declare_dram_parameter("packb", [64, 128], bf16, isOutput=False)
    sel2 = nc.declare_dram_parameter("sel2", [2, 128], bf16, isOutput=False)
    ident = nc.declare_dram_parameter("ident", [128, 128], bf16, isOutput=False)
    # mask[kt_local, i*256 + blk*128 + q_local] in {0, 1}
    maskT = nc.declare_dram_parameter("maskT", [128, NT, QT], bf16, isOutput=False)
    out = nc.declare_dram_parameter("out", [QT, D_MODEL], bf16, isOutput=True)

    # ---- internal DRAM for the group collectives ----
    kch = nc.dram_tensor("kch", [128, S], bf16, kind="Internal")
    kall = nc.dram_tensor("kall", [4, 128, S], bf16, kind="Internal")
    vch = nc.dram_tensor("vch", [S, 128], bf16, kind="Internal")
    vall = nc.dram_tensor("vall", [4, S, 128], bf16, kind="Internal")

    with tile.TileContext(nc) as tc:
        import contextlib

        with contextlib.ExitStack() as es:
            singles = es.enter_context(tc.tile_pool(name="singles", bufs=1))

            # ---- persistent constants / tables ----
            ck_sb = singles.tile([128, S], bf16)
            dk_sb = singles.tile([128, S], bf16)
            cq_sb = singles.tile([128, QT], bf16)
            dq_sb = singles.tile([128, QT], bf16)
            psw_sb = singles.tile([128, 128], bf16)
            pka_sb = singles.tile([64, 128], bf16)
            pkb_sb = singles.tile([64, 128], bf16)
            sel2_sb = singles.tile([2, 128], bf16)
            id_sb = singles.tile([128, 128], bf16)
            mask_sb = singles.tile([128, NT, QT], bf16)
            bq_sb = singles.tile([128, 16], mybir.dt.float32)
            bk_sb = singles.tile([128, 1], mybir.dt.float32)
            bv_sb = singles.tile([128, 1], mybir.dt.float32)
            bo_sb = singles.tile([1, D_MODEL], bf16)
            ones_row = singles.tile([1, 128], bf16)
            nc.vector.memset(ones_row, 1.0)
            for t, src in [
                (ck_sb, ckt), (dk_sb, dkt), (cq_sb, cqt), (dq_sb, dqt),
                (psw_sb, pswap), (pka_sb, packa), (pkb_sb, packb),
                (sel2_sb, sel2), (id_sb, ident), (mask_sb, maskT),
                (bq_sb, bq), (bk_sb, bk), (bv_sb, bv), (bo_sb, bo),
            ]:
                nc.sync.dma_start(out=t, in_=src[:])

            # ---- persistent activations ----
            ropek = [singles.tile([64, S], bf16, name=f"ropek{i}", tag=f"ropek{i}") for i in range(N_KV)]
            ropeq = [singles.tile([64, QT], bf16, name=f"ropeq{i}", tag=f"ropeq{i}") for i in range(N_HEAD)]
            vp = [singles.tile([128, NT, 65], bf16, name=f"vp{i}", tag=f"vp{i}") for i in range(N_KV)]
            o_sb = [singles.tile([65, QT], bf16, name=f"osb{i}", tag=f"osb{i}") for i in range(N_HEAD)]
            attT = [singles.tile([128, QT], bf16, name=f"attT{i}", tag=f"attT{i}") for i in range(N_HEAD // 2)]
            den_lo = singles.tile([16, QT], bf16)
            den_hi = singles.tile([16, QT], bf16)
            rden_lo = singles.tile([16, QT], bf16)
            rden_hi = singles.tile([16, QT], bf16)
            rden2 = singles.tile([2, 16, QT], bf16)

            # =========== Phase 1: projections + rope + K/V gather ===========
            with contextlib.ExitStack() as proj_es:
                ppool = proj_es.enter_context(tc.tile_pool(name="proj", bufs=1))
                wqpool = proj_es.enter_context(tc.tile_pool(name="wqp", bufs=3))
                pwork = proj_es.enter_context(tc.tile_pool(name="pwork", bufs=4))
                psA = proj_es.enter_context(tc.tile_pool(name="psA", bufs=2, space="PSUM"))
                psB = proj_es.enter_context(tc.tile_pool(name="psB", bufs=2, space="PSUM"))

                wk_sb = ppool.tile([128, 16, 128], bf16)
                wv_sb = ppool.tile([128, 16, 128], bf16)
                xT_sb = ppool.tile([128, 16, S], bf16)
                xq_sb = ppool.tile([128, 16, QT], bf16)
                nc.sync.dma_start(out=wk_sb, in_=wk[:])
                nc.sync.dma_start(out=wv_sb, in_=wv[:])
                for g in range(4):
                    nc.sync.dma_start(out=xT_sb[:, 4 * g:4 * g + 4, :],
                                      in_=xT[:, 4 * g:4 * g + 4, :])
                nc.sync.dma_start(out=xq_sb, in_=xq[:])

                # ---- K^T chunk (2 kv heads) = wk^T @ x^T, bias, rope ----
                kps = psA.tile([128, S], mybir.dt.float32, tag="A")
                for hf in range(2):
                    cols = slice(hf * 512, hf * 512 + 512)
                    for kk in range(16):
                        nc.tensor.matmul(
                            kps[:, cols],
                            wk_sb[:, kk, :],
                            xT_sb[:, kk, cols],
                            start=(kk == 0), stop=(kk == 15),
                        )
                k_sb = pwork.tile([128, S], bf16, tag="ksb")
                nc.scalar.add(k_sb, kps, bk_sb)
                swp = psB.tile([128, S], mybir.dt.float32, tag="B")
                for hf in range(2):
                    cols = slice(hf * 512, hf * 512 + 512)
                    nc.tensor.matmul(swp[:, cols], psw_sb, k_sb[:, cols],
                                     start=True, stop=True)
                t1 = pwork.tile([128, S], bf16, tag="t1")
                t2 = pwork.tile([128, S], bf16, tag="t2")
                kch_sb = pwork.tile([128, S], bf16, tag="kch")
                nc.gpsimd.tensor_mul(t1, k_sb, ck_sb)
                nc.vector.tensor_mul(t2, swp, dk_sb)
                nc.vector.tensor_add(kch_sb, t1, t2)
                nc.sync.dma_start(out=kch[:], in_=kch_sb)
                nc.gpsimd.collective_compute(
                    "AllGather", mybir.AluOpType.bypass,
                    replica_groups=GROUPS,
                    ins=[kch[:].opt()], outs=[kall[:].opt()],
                )

                # ---- V^T chunk (2 kv heads), bias, transpose, gather ----
                vps = psA.tile([128, S], mybir.dt.float32, tag="A")
                for hf in range(2):
                    cols = slice(hf * 512, hf * 512 + 512)
                    for kk in range(16):
                        nc.tensor.matmul(
                            vps[:, cols],
                            wv_sb[:, kk, :],
                            xT_sb[:, kk, cols],
                            start=(kk == 0), stop=(kk == 15),
                        )
                vT_sb = pwork.tile([128, S], bf16, tag="vts")
                nc.scalar.add(vT_sb, vps, bv_sb)
                vtp = psB.tile([128, NT, 128], bf16, tag="B")
                for i in range(NT):
                    nc.tensor.transpose(vtp[:, i, :], vT_sb[:, i * 128:i * 128 + 128], id_sb)
                vt_sb = pwork.tile([128, NT, 128], bf16, tag="vtsb")
                nc.vector.tensor_copy(vt_sb, vtp)
                nc.sync.dma_start(
                    out=vch[:].rearrange("(i p) c -> p i c", p=128),
                    in_=vt_sb,
                )
                nc.gpsimd.collective_compute(
                    "AllGather", mybir.AluOpType.bypass,
                    replica_groups=GROUPS,
                    ins=[vch[:].opt()], outs=[vall[:].opt()],
                )

                # ---- Q^T = wq^T @ xq^T, bias, rope -> ropeq[64, QT] ----
                for m in range(16):
                    wqm = wqpool.tile([128, 16, 128], bf16, tag="wqm")
                    nc.sync.dma_start(out=wqm, in_=wq[:, m, :, :])
                    qps = psA.tile([128, QT], mybir.dt.float32, tag="A")
                    for kk in range(16):
                        nc.tensor.matmul(
                            qps,
                            wqm[:, kk, :],
                            xq_sb[:, kk, :],
                            start=(kk == 0), stop=(kk == 15),
                        )
                    q_sb = pwork.tile([128, QT], bf16, tag="qsb")
                    nc.scalar.add(q_sb, qps, bq_sb[:, m:m + 1])
                    swq = psB.tile([128, QT], mybir.dt.float32, tag="B")
                    nc.tensor.matmul(swq, psw_sb, q_sb, start=True, stop=True)
                    t1q = pwork.tile([128, QT], bf16, tag="qt1")
                    t2q = pwork.tile([128, QT], bf16, tag="qt2")
                    nc.gpsimd.tensor_mul(t1q, q_sb, cq_sb)
                    nc.vector.tensor_mul(t2q, swq, dq_sb)
                    nc.vector.tensor_add(ropeq[2 * m], t1q[0:64, :], t2q[0:64, :])
                    nc.vector.tensor_add(ropeq[2 * m + 1], t1q[64:128, :], t2q[64:128, :])

            # ---- land gathered K/V into SBUF ----
            for h in range(N_KV):
                nc.sync.dma_start(
                    out=ropek[h],
                    in_=kall[h // 2, 64 * (h % 2):64 * (h % 2) + 64, :],
                )
                nc.vector.memset(vp[h], 1.0)
                nc.sync.dma_start(
                    out=vp[h][:, :, 0:64],
                    in_=vall[h // 2].rearrange("(i p) c -> p i c", p=128)[
                        :, :, 64 * (h % 2):64 * (h % 2) + 64],
                )

            # =========== Phase 2: attention ===========
            with contextlib.ExitStack() as att_es:
                apool = att_es.enter_context(tc.tile_pool(name="att", bufs=3))
                psA2 = att_es.enter_context(tc.tile_pool(name="psA2", bufs=2, space="PSUM"))
                psS2 = att_es.enter_context(tc.tile_pool(name="psS2", bufs=2, space="PSUM"))
                psB2 = att_es.enter_context(tc.tile_pool(name="psB2", bufs=2, space="PSUM"))

                def norm_pair(pr):
                    # broadcast 1/den over each head's 64 rows, normalize in place
                    bc = psS2.tile([128, QT], mybir.dt.float32, tag="S")
                    nc.tensor.matmul(bc, sel2_sb, rden2[:, pr, :],
                                     start=True, stop=True)
                    nc.vector.tensor_mul(attT[pr], attT[pr], bc)

                for h in range(N_HEAD):
                    kvh = h // N_REP
                    # scores: tiles 0-3 for both blocks, 4-7 for block B only
                    sA = psA2.tile([128, 4, QT], mybir.dt.float32, tag="A")
                    for i in range(4):
                        nc.tensor.matmul(
                            sA[:, i, :],
                            ropek[kvh][:, i * 128:i * 128 + 128],
                            ropeq[h],
                            start=True, stop=True,
                        )
                    sB = psS2.tile([128, 4, 128], mybir.dt.float32, tag="S")
                    for i in range(4, 8):
                        nc.tensor.matmul(
                            sB[:, i - 4, :],
                            ropek[kvh][:, i * 128:i * 128 + 128],
                            ropeq[h][:, 128:256],
                            start=True, stop=True,
                        )
                    probs = apool.tile([128, NT, QT], bf16, tag="probs")
                    if h < 3:
                        nc.vector.memset(probs, 0.0)
                    nc.scalar.activation(
                        probs[:, 0:4, :], sA,
                        mybir.ActivationFunctionType.Exp,
                        bias=0.0, scale=0.125,
                    )
                    nc.scalar.activation(
                        probs[:, 4:8, 128:256], sB,
                        mybir.ActivationFunctionType.Exp,
                        bias=0.0, scale=0.125,
                    )
                    nc.vector.tensor_mul(probs, probs, mask_sb)
                    outv = psB2.tile([65, QT], mybir.dt.float32, tag="B")
                    for i in range(NT):
                        nc.tensor.matmul(
                            outv,
                            vp[kvh][:, i, :],
                            probs[:, i, :],
                            start=(i == 0), stop=(i == NT - 1),
                        )
                    nc.vector.tensor_copy(o_sb[h], outv)
                    dtile = den_lo if h < 16 else den_hi
                    nc.sync.dma_start(out=dtile[h % 16:h % 16 + 1, :],
                                      in_=o_sb[h][64:65, :])
                    if h % 2 == 1:
                        pr = h // 2
                        pk = psB2.tile([128, QT], mybir.dt.float32, tag="B")
                        nc.tensor.matmul(pk, pka_sb, o_sb[h - 1][0:64, :],
                                         start=True, stop=False)
                        nc.tensor.matmul(pk, pkb_sb, o_sb[h][0:64, :],
                                         start=False, stop=True)
                        nc.vector.tensor_copy(attT[pr], pk)
                    if h == 16:
                        # first half's denominators are in; batch their recip
                        with nc.allow_low_precision(reason="1/den in bf16 is plenty"):
                            nc.vector.reciprocal(rden_lo, den_lo)
                        for pr in range(8):
                            nc.sync.dma_start(out=rden2[:, pr, :],
                                              in_=rden_lo[2 * pr:2 * pr + 2, :])
                    if h >= 17 and h % 2 == 1:
                        norm_pair((h - 17) // 2)
                with nc.allow_low_precision(reason="1/den in bf16 is plenty"):
                    nc.vector.reciprocal(rden_hi, den_hi)
                for pr in range(8, 16):
                    nc.sync.dma_start(out=rden2[:, pr, :],
                                      in_=rden_hi[2 * (pr - 8):2 * (pr - 8) + 2, :])
                for pr in range(8, 16):
                    norm_pair(pr)

            # =========== Phase 3: output projection ===========
            with contextlib.ExitStack() as op_es:
                wopool = op_es.enter_context(tc.tile_pool(name="wop", bufs=2))
                opool = op_es.enter_context(tc.tile_pool(name="osb", bufs=3))
                psO = op_es.enter_context(tc.tile_pool(name="psO", bufs=4, space="PSUM"))
                for n in range(4):
                    won = wopool.tile([128, 16, 512], bf16, tag="won")
                    nc.sync.dma_start(out=won, in_=wo[:, n, :, :])
                    for blk in range(2):
                        ops = psO.tile([128, 512], mybir.dt.float32, tag="O")
                        for p in range(16):
                            nc.tensor.matmul(
                                ops,
                                attT[p][:, blk * 128:blk * 128 + 128],
                                won[:, p, :],
                                start=(p == 0), stop=False,
                            )
                        nc.tensor.matmul(
                            ops,
                            ones_row,
                            bo_sb[:, n * 512:n * 512 + 512],
                            start=False, stop=True,
                        )
                        osb = opool.tile([128, 512], bf16, tag="osb")
                        nc.scalar.copy(osb, ops)
                        nc.sync.dma_start(
                            out=out[blk * 128:blk * 128 + 128, n * 512:n * 512 + 512],
                            in_=osb,
                        )
    return nc


def _host_prep(x, Wq, bq, Wk, bk, Wv, bv, Wo, bo):
    """Build per-core input maps."""
    # per-head even/odd deinterleave permutation of output columns
    def colperm(nheads):
        p = []
        for h in range(nheads):
            base = h * DK
            p.extend([base + 2 * j for j in range(HALF)])
            p.extend([base + 2 * j + 1 for j in range(HALF)])
        return np.array(p)

    qperm = colperm(N_HEAD)
    kperm = colperm(N_KV)
    wq_p = np.asarray(Wq, np.float32)[:, qperm]
    wk_p = np.asarray(Wk, np.float32)[:, kperm]
    bq_p = np.asarray(bq, np.float32)[qperm]
    bk_p = np.asarray(bk, np.float32)[kperm]
    wv_c = np.asarray(Wv, np.float32)
    bv_c = np.asarray(bv, np.float32)
    wo_c = np.asarray(Wo, np.float32)

    wq_pre = np.ascontiguousarray(
        wq_p.reshape(16, 128, 16, 128).transpose(1, 2, 0, 3)).astype(BF16)
    wo_pre = np.ascontiguousarray(
        wo_c.reshape(16, 128, 4, 512).transpose(1, 2, 0, 3)).astype(BF16)
    bq_pre = np.ascontiguousarray(bq_p.reshape(16, 128).T).astype(np.float32)
    bo_r = np.asarray(bo, np.float32).astype(BF16).reshape(1, D_MODEL)

    invf = THETA ** (-(np.arange(HALF, dtype=np.float64) * 2.0 / DK))
    posf = np.arange(S, dtype=np.float64)
    ang = posf[:, None] * invf[None, :]  # [S, 32]
    cos_t, sin_t = np.cos(ang), np.sin(ang)

    def rope_tables(pos_idx):
        n = len(pos_idx)
        C = np.zeros((128, n), np.float32)
        D = np.zeros((128, n), np.float32)
        for p in range(128):
            r = p % DK
            i = r if r < HALF else r - HALF
            C[p] = cos_t[pos_idx, i]
            D[p] = (-sin_t[pos_idx, i]) if r < HALF else sin_t[pos_idx, i]
        return C.astype(BF16), D.astype(BF16)

    ckt, dkt = rope_tables(np.arange(S))

    psw = np.zeros((128, 128), np.float32)
    for m in range(128):
        k = m + HALF if (m % DK) < HALF else m - HALF
        psw[k, m] = 1.0
    psw = psw.astype(BF16)
    pka = np.zeros((64, 128), np.float32)
    pkb = np.zeros((64, 128), np.float32)
    for k in range(64):
        pka[k, k] = 1.0
        pkb[k, k + 64] = 1.0
    pka, pkb = pka.astype(BF16), pkb.astype(BF16)
    sel2 = np.zeros((2, 128), np.float32)
    sel2[0, 0:64] = 1.0
    sel2[1, 64:128] = 1.0
    sel2 = sel2.astype(BF16)
    ident = np.eye(128, dtype=np.float32).astype(BF16)

    in_maps = []
    meta = []
    for c in range(8):
        b, r = c // 4, c % 4
        blocks = [r, 7 - r]
        qrows = np.concatenate([np.arange(bb * 128, bb * 128 + 128) for bb in blocks])
        xb = np.asarray(x[b], dtype=np.float32)
        xT_pre = np.ascontiguousarray(
            xb.reshape(S, 16, 128).transpose(2, 1, 0)).astype(BF16)
        xq_pre = np.ascontiguousarray(
            xb[qrows].reshape(QT, 16, 128).transpose(2, 1, 0)).astype(BF16)
        wk_sh = np.ascontiguousarray(
            wk_p[:, 128 * r:128 * r + 128].reshape(16, 128, 128).transpose(1, 0, 2)
        ).astype(BF16)
        wv_sh = np.ascontiguousarray(
            wv_c[:, 128 * r:128 * r + 128].reshape(16, 128, 128).transpose(1, 0, 2)
        ).astype(BF16)
        bk_sh = np.ascontiguousarray(bk_p[128 * r:128 * r + 128].reshape(128, 1))
        bv_sh = np.ascontiguousarray(bv_c[128 * r:128 * r + 128].reshape(128, 1))
        cqt, dqt = rope_tables(qrows)
        # mask[kt_local, i, blk*128 + ql] = 1 if (i*128+kt_local) <= qpos else 0
        mask = np.zeros((128, NT, QT), np.float32)
        kt_local = np.arange(128)
        for i in range(NT):
            ktg = i * 128 + kt_local
            for blki, bb in enumerate(blocks):
                qpos = bb * 128 + np.arange(128)
                mask[:, i, blki * 128:blki * 128 + 128] = (
                    ktg[:, None] <= qpos[None, :]
                )
        in_maps.append({
            "xT": xT_pre, "xq": xq_pre, "wq": wq_pre, "wk": wk_sh, "wv": wv_sh,
            "wo": wo_pre,
            "bq": bq_pre, "bk": bk_sh, "bv": bv_sh, "bo": bo_r,
            "ckt": ckt, "dkt": dkt, "cqt": cqt, "dqt": dqt,
            "pswap": psw, "packa": pka, "packb": pkb, "sel2": sel2,
            "ident": ident,
            "maskT": mask.astype(BF16),
        })
        meta.append((b, blocks))
    return in_maps, meta


def kernel(x, Wq, bq, Wk, bk, Wv, bv, Wo, bo):
    if "nc" not in _cache:
        nc0 = _build_nc()
        nc0.finalize()
        _cache["nc"] = nc0
    nc = _cache["nc"]
    in_maps, meta = _host_prep(x, Wq, bq, Wk, bk, Wv, bv, Wo, bo)
    res = run_bass_kernel_spmd(nc, in_maps, list(range(8)))
    full = np.zeros((B, S, D_MODEL), np.float32)
    for c in range(8):
        b, blocks = meta[c]
        o = np.asarray(res.results[c]["out"], dtype=np.float32)
        for blki, bb in enumerate(blocks):
            full[b, bb * 128:bb * 128 + 128] = o[blki * 128:(blki + 1) * 128]
    return full


# revision 15
# speedup vs baseline: 1.3645x; 1.0834x over previous
"""GQA attention (B=2, S=1024, D=2048, 32 q heads / 8 kv heads, RoPE, causal)
on 8 TRN2 NeuronCores.

Strategy (v2): data parallel on batch (4 cores per batch), with the K/V
projections tensor-parallel *within* each batch group: core with group rank r
computes kv heads {2r, 2r+1} only (1/4 of the old replicated work), ropes K
locally, and the group AllGathers the K^T / V chunks through HBM while the
(long) Q projection runs.  Attention itself stays data-parallel: core r
handles q-token blocks {r, 7-r} of its batch (9 causal kv-tiles of work for
every core).  Causal structure is exploited uniformly: block A = r < 4 never
attends past kv tile 3, so score tiles 4-7 are computed for block B only
(25% fewer score columns + exp work); the multiplicative mask (per-core DATA)
zeroes everything invalid, so the SPMD instruction stream is identical on all
cores.

Softmax denominators are batched: each head's denominator row (from the ones
column appended to V) is copied to one partition of a [32, 256] tile via
SBUF->SBUF DMA, a single reciprocal + tiny broadcast matmuls replace 32
single-partition reciprocals.  Bias adds ride the scalar engine's PSUM->SBUF
copy (per-partition bias) for Q/K/V^T.  Wq/Wo are pre-swizzled on the host so
all weight DMAs are contiguous.
"""

import numpy as np
import ml_dtypes

import concourse.bass as bass
import concourse.tile as tile
from concourse import bacc
from concourse import mybir
from concourse.bass_utils import run_bass_kernel_spmd

BF16 = ml_dtypes.bfloat16
D_MODEL = 2048
N_HEAD = 32
N_KV = 8
N_REP = 4
DK = 64
HALF = 32
THETA = 10000.0
B, S = 2, 1024
NT = S // 128  # 8 kv tiles of 128
QT = 256  # q tokens per core (two blocks of 128)
GROUPS = [[0, 1, 2, 3], [4, 5, 6, 7]]

_cache = {}


def _build_nc():
    nc = bacc.Bacc("TRN2", target_bir_lowering=False, debug=False)
    f32 = mybir.dt.float32
    bf16 = mybir.dt.bfloat16

    # ---- DRAM parameters (per-core shards supplied via in_maps) ----
    xT = nc.declare_dram_parameter("xT", [128, 16, S], bf16, isOutput=False)
    xq = nc.declare_dram_parameter("xq", [128, 16, QT], bf16, isOutput=False)
    wq = nc.declare_dram_parameter("wq", [128, 16, 16, 128], bf16, isOutput=False)
    wk = nc.declare_dram_parameter("wk", [128, 16, 128], bf16, isOutput=False)
    wv = nc.declare_dram_parameter("wv", [128, 16, 128], bf16, isOutput=False)
    wo = nc.declare_dram_parameter("wo", [128, 4, 16, 512], bf16, isOutput=False)
    bq = nc.declare_dram_parameter("bq", [128, 16], f32, isOutput=False)
    bk = nc.declare_dram_parameter("bk", [128, 1], f32, isOutput=False)
    bv = nc.declare_dram_parameter("bv", [128, 1], f32, isOutput=False)
    bo = nc.declare_dram_parameter("bo", [1, D_MODEL], bf16, isOutput=False)
    ckt = nc.declare_dram_parameter("ckt", [128, S], bf16, isOutput=False)
    dkt = nc.declare_dram_parameter("dkt", [128, S], bf16, isOutput=False)
    cqt = nc.declare_dram_parameter("cqt", [128, QT], bf16, isOutput=False)
    dqt = nc.declare_dram_parameter("dqt", [128, QT], bf16, isOutput=False)
    pswap = nc.declare_dram_parameter("pswap", [128, 128], bf16, isOutput=False)
    packa = nc.declare_dram_parameter("packa", [64, 128], bf16, isOutput=False)
    packb = nc.declare_dram_parameter("packb", [64, 128], bf16, isOutput=False)
    sel2 = nc.declare_dram_parameter("sel2", [2, 128], bf16, isOutput=False)
    ident = nc.declare_dram_parameter("ident", [128, 128], bf16, isOutput=False)
    # mask[kt_local, i*256 + blk*128 + q_local] in {0, 1}
    maskT = nc.declare_dram_parameter("maskT", [128, NT, QT], bf16, isOutput=False)
    out = nc.declare_dram_parameter("out", [QT, D_MODEL], bf16, isOutput=True)

    # ---- internal DRAM for the group collective (K and V in one buffer) ----
    kvch = nc.dram_tensor("kvch", [2, 128 * S], bf16, kind="Internal")
    kvall = nc.dram_tensor("kvall", [4, 2, 128 * S], bf16, kind="Internal")

    with tile.TileContext(nc) as tc:
        import contextlib

        with contextlib.ExitStack() as es:
            singles = es.enter_context(tc.tile_pool(name="singles", bufs=1))

            # ---- persistent constants / tables ----
            ck_sb = singles.tile([128, S], bf16)
            dk_sb = singles.tile([128, S], bf16)
            cq_sb = singles.tile([128, QT], bf16)
            dq_sb = singles.tile([128, QT], bf16)
            psw_sb = singles.tile([128, 128], bf16)
            pka_sb = singles.tile([64, 128], bf16)
            pkb_sb = singles.tile([64, 128], bf16)
            sel2_sb = singles.tile([2, 128], bf16)
            id_sb = singles.tile([128, 128], bf16)
            mask_sb = singles.tile([128, NT, QT], bf16)
            bq_sb = singles.tile([128, 16], mybir.dt.float32)
            bk_sb = singles.tile([128, 1], mybir.dt.float32)
            bv_sb = singles.tile([128, 1], mybir.dt.float32)
            bo_sb = singles.tile([1, D_MODEL], bf16)
            ones_row = singles.tile([1, 128], bf16)
            nc.vector.memset(ones_row, 1.0)


            # ---- persistent activations ----
            ropek = [singles.tile([64, S], bf16, name=f"ropek{i}", tag=f"ropek{i}") for i in range(N_KV)]
            ropeq = [singles.tile([64, QT], bf16, name=f"ropeq{i}", tag=f"ropeq{i}") for i in range(N_HEAD)]
            vp = [singles.tile([128, NT, 65], bf16, name=f"vp{i}", tag=f"vp{i}") for i in range(N_KV)]
            o_sb = [singles.tile([65, QT], bf16, name=f"osb{i}", tag=f"osb{i}") for i in range(N_HEAD)]
            attT = [singles.tile([128, QT], bf16, name=f"attT{i}", tag=f"attT{i}") for i in range(N_HEAD // 2)]
            den_lo = singles.tile([16, QT], bf16)
            den_hi = singles.tile([16, QT], bf16)
            rden_lo = singles.tile([16, QT], bf16)
            rden_hi = singles.tile([16, QT], bf16)
            rden2 = singles.tile([2, 16, QT], bf16)

            # =========== Phase 1: projections + rope + K/V gather ===========
            with contextlib.ExitStack() as proj_es:
                ppool = proj_es.enter_context(tc.tile_pool(name="proj", bufs=1))
                wqpool = proj_es.enter_context(tc.tile_pool(name="wqp", bufs=3))
                pwork = proj_es.enter_context(tc.tile_pool(name="pwork", bufs=2))
                psA = proj_es.enter_context(tc.tile_pool(name="psA", bufs=2, space="PSUM"))
                psB = proj_es.enter_context(tc.tile_pool(name="psB", bufs=2, space="PSUM"))

                wk_sb = ppool.tile([128, 16, 128], bf16)
                wv_sb = ppool.tile([128, 16, 128], bf16)
                xT_sb = ppool.tile([128, 16, S], bf16)
                xq_sb = ppool.tile([128, 16, QT], bf16)
                nc.sync.dma_start(out=wk_sb, in_=wk[:])
                nc.sync.dma_start(out=wv_sb, in_=wv[:])
                for g in range(4):
                    nc.sync.dma_start(out=xT_sb[:, 4 * g:4 * g + 4, :],
                                      in_=xT[:, 4 * g:4 * g + 4, :])
                # tables, ordered by first use: K-rope, V/ident, then Q-side
                for t, tsrc in [
                    (ck_sb, ckt), (dk_sb, dkt), (bk_sb, bk), (psw_sb, pswap),
                    (bv_sb, bv), (id_sb, ident),
                ]:
                    nc.sync.dma_start(out=t, in_=tsrc[:])
                nc.sync.dma_start(out=xq_sb, in_=xq[:])
                for t, tsrc in [
                    (bq_sb, bq), (cq_sb, cqt), (dq_sb, dqt),
                    (pka_sb, packa), (pkb_sb, packb), (sel2_sb, sel2),
                    (mask_sb, maskT), (bo_sb, bo),
                ]:
                    nc.sync.dma_start(out=t, in_=tsrc[:])

                # ---- K^T chunk (2 kv heads) = wk^T @ x^T, bias, rope ----
                kps = psA.tile([128, S], mybir.dt.float32, tag="A")
                for hf in range(2):
                    cols = slice(hf * 512, hf * 512 + 512)
                    for kk in range(16):
                        nc.tensor.matmul(
                            kps[:, cols],
                            wk_sb[:, kk, :],
                            xT_sb[:, kk, cols],
                            start=(kk == 0), stop=(kk == 15),
                        )
                k_sb = pwork.tile([128, S], bf16, tag="ksb")
                nc.scalar.add(k_sb, kps, bk_sb)
                swp = psB.tile([128, S], mybir.dt.float32, tag="B")
                for hf in range(2):
                    cols = slice(hf * 512, hf * 512 + 512)
                    nc.tensor.matmul(swp[:, cols], psw_sb, k_sb[:, cols],
                                     start=True, stop=True)
                t1 = pwork.tile([128, S], bf16, tag="t1")
                t2 = pwork.tile([128, S], bf16, tag="t2")
                kch_sb = pwork.tile([128, S], bf16, tag="kch")
                nc.gpsimd.tensor_mul(t1, k_sb, ck_sb)
                nc.vector.tensor_mul(t2, swp, dk_sb)
                nc.vector.tensor_add(kch_sb, t1, t2)
                nc.sync.dma_start(
                    out=kvch[0].rearrange("(p s) -> p s", p=128), in_=kch_sb)

                # ---- V^T chunk (2 kv heads), bias, transpose, gather ----
                vps = psA.tile([128, S], mybir.dt.float32, tag="A")
                for hf in range(2):
                    cols = slice(hf * 512, hf * 512 + 512)
                    for kk in range(16):
                        nc.tensor.matmul(
                            vps[:, cols],
                            wv_sb[:, kk, :],
                            xT_sb[:, kk, cols],
                            start=(kk == 0), stop=(kk == 15),
                        )
                vT_sb = pwork.tile([128, S], bf16, tag="vts")
                nc.scalar.add(vT_sb, vps, bv_sb)
                vtp = psB.tile([128, NT, 128], bf16, tag="B")
                for i in range(NT):
                    nc.tensor.transpose(vtp[:, i, :], vT_sb[:, i * 128:i * 128 + 128], id_sb)
                vt_sb = pwork.tile([128, NT, 128], bf16, tag="vtsb")
                nc.vector.tensor_copy(vt_sb, vtp)
                nc.sync.dma_start(
                    out=kvch[1].rearrange("(i p c) -> p i c", p=128, c=128),
                    in_=vt_sb,
                )
                nc.gpsimd.collective_compute(
                    "AllGather", mybir.AluOpType.bypass,
                    replica_groups=GROUPS,
                    ins=[kvch[:].opt()], outs=[kvall[:].opt()],
                )

                # ---- Q^T = wq^T @ xq^T, bias, rope -> ropeq[64, QT] ----
                for m in range(16):
                    wqm = wqpool.tile([128, 16, 128], bf16, tag="wqm")
                    nc.sync.dma_start(out=wqm, in_=wq[:, m, :, :])
                    qps = psA.tile([128, QT], mybir.dt.float32, tag="A")
                    for kk in range(16):
                        nc.tensor.matmul(
                            qps,
                            wqm[:, kk, :],
                            xq_sb[:, kk, :],
                            start=(kk == 0), stop=(kk == 15),
                        )
                    q_sb = pwork.tile([128, QT], bf16, tag="qsb")
                    nc.scalar.add(q_sb, qps, bq_sb[:, m:m + 1])
                    swq = psB.tile([128, QT], mybir.dt.float32, tag="B")
                    nc.tensor.matmul(swq, psw_sb, q_sb, start=True, stop=True)
                    t1q = pwork.tile([128, QT], bf16, tag="qt1")
                    t2q = pwork.tile([128, QT], bf16, tag="qt2")
                    nc.gpsimd.tensor_mul(t1q, q_sb, cq_sb)
                    nc.vector.tensor_mul(t2q, swq, dq_sb)
                    nc.vector.tensor_add(ropeq[2 * m], t1q[0:64, :], t2q[0:64, :])
                    nc.vector.tensor_add(ropeq[2 * m + 1], t1q[64:128, :], t2q[64:128, :])

            # ---- land gathered K/V into SBUF ----
            for h in range(N_KV):
                nc.sync.dma_start(
                    out=ropek[h],
                    in_=kvall[h // 2, 0].rearrange("(p s) -> p s", p=128)[
                        64 * (h % 2):64 * (h % 2) + 64, :],
                )
                nc.vector.memset(vp[h], 1.0)
                nc.sync.dma_start(
                    out=vp[h][:, :, 0:64],
                    in_=kvall[h // 2, 1].rearrange("(i p c) -> p i c", p=128, c=128)[
                        :, :, 64 * (h % 2):64 * (h % 2) + 64],
                )

            # =========== Phase 2: attention ===========
            wopool = es.enter_context(tc.tile_pool(name="wop", bufs=2))
            wons = []
            for n in range(2):
                won = wopool.tile([128, 16, 512], bf16, tag="won")
                nc.sync.dma_start(out=won, in_=wo[:, n, :, :])
                wons.append(won)
            with contextlib.ExitStack() as att_es:
                apool = att_es.enter_context(tc.tile_pool(name="att", bufs=3))
                psA2 = att_es.enter_context(tc.tile_pool(name="psA2", bufs=2, space="PSUM"))
                psS2 = att_es.enter_context(tc.tile_pool(name="psS2", bufs=2, space="PSUM"))
                psB2 = att_es.enter_context(tc.tile_pool(name="psB2", bufs=2, space="PSUM"))

                def norm_pair(pr):
                    # broadcast 1/den over each head's 64 rows, normalize in place
                    bc = psS2.tile([128, QT], mybir.dt.float32, tag="S")
                    nc.tensor.matmul(bc, sel2_sb, rden2[:, pr, :],
                                     start=True, stop=True)
                    nc.vector.tensor_mul(attT[pr], attT[pr], bc)

                for h in range(N_HEAD):
                    kvh = h // N_REP
                    # scores: tiles 0-3 for both blocks, 4-7 for block B only
                    sA = psA2.tile([128, 4, QT], mybir.dt.float32, tag="A")
                    for i in range(4):
                        nc.tensor.matmul(
                            sA[:, i, :],
                            ropek[kvh][:, i * 128:i * 128 + 128],
                            ropeq[h],
                            start=True, stop=True,
                        )
                    sB = psS2.tile([128, 4, 128], mybir.dt.float32, tag="S")
                    for i in range(4, 8):
                        nc.tensor.matmul(
                            sB[:, i - 4, :],
                            ropek[kvh][:, i * 128:i * 128 + 128],
                            ropeq[h][:, 128:256],
                            start=True, stop=True,
                        )
                    probs = apool.tile([128, NT, QT], bf16, tag="probs")
                    if h < 3:
                        nc.vector.memset(probs, 0.0)
                    nc.scalar.activation(
                        probs[:, 0:4, :], sA,
                        mybir.ActivationFunctionType.Exp,
                        bias=0.0, scale=0.125,
                    )
                    nc.scalar.activation(
                        probs[:, 4:8, 128:256], sB,
                        mybir.ActivationFunctionType.Exp,
                        bias=0.0, scale=0.125,
                    )
                    nc.vector.tensor_mul(probs, probs, mask_sb)
                    outv = psB2.tile([65, QT], mybir.dt.float32, tag="B")
                    for i in range(NT):
                        nc.tensor.matmul(
                            outv,
                            vp[kvh][:, i, :],
                            probs[:, i, :],
                            start=(i == 0), stop=(i == NT - 1),
                        )
                    nc.vector.tensor_copy(o_sb[h], outv)
                    dtile = den_lo if h < 16 else den_hi
                    nc.sync.dma_start(out=dtile[h % 16:h % 16 + 1, :],
                                      in_=o_sb[h][64:65, :])
                    if h % 2 == 1:
                        pr = h // 2
                        pk = psB2.tile([128, QT], mybir.dt.float32, tag="B")
                        nc.tensor.matmul(pk, pka_sb, o_sb[h - 1][0:64, :],
                                         start=True, stop=False)
                        nc.tensor.matmul(pk, pkb_sb, o_sb[h][0:64, :],
                                         start=False, stop=True)
                        nc.vector.tensor_copy(attT[pr], pk)
                    if h == 16:
                        # first half's denominators are in; batch their recip
                        with nc.allow_low_precision(reason="1/den in bf16 is plenty"):
                            nc.vector.reciprocal(rden_lo, den_lo)
                        for pr in range(8):
                            nc.sync.dma_start(out=rden2[:, pr, :],
                                              in_=rden_lo[2 * pr:2 * pr + 2, :])
                    if h >= 17 and h % 2 == 1:
                        norm_pair((h - 17) // 2)
                with nc.allow_low_precision(reason="1/den in bf16 is plenty"):
                    nc.vector.reciprocal(rden_hi, den_hi)
                for pr in range(8, 16):
                    nc.sync.dma_start(out=rden2[:, pr, :],
                                      in_=rden_hi[2 * (pr - 8):2 * (pr - 8) + 2, :])
                for pr in range(8, 16):
                    norm_pair(pr)

            # =========== Phase 3: output projection ===========
            with contextlib.ExitStack() as op_es:
                opool = op_es.enter_context(tc.tile_pool(name="osb", bufs=3))
                psO = op_es.enter_context(tc.tile_pool(name="psO", bufs=4, space="PSUM"))
                for n in range(4):
                    if n < 2:
                        won = wons[n]
                    else:
                        won = wopool.tile([128, 16, 512], bf16, tag="won")
                        nc.sync.dma_start(out=won, in_=wo[:, n, :, :])
                    for blk in range(2):
                        ops = psO.tile([128, 512], mybir.dt.float32, tag="O")
                        for p in range(16):
                            nc.tensor.matmul(
                                ops,
                                attT[p][:, blk * 128:blk * 128 + 128],
                                won[:, p, :],
                                start=(p == 0), stop=False,
                            )
                        nc.tensor.matmul(
                            ops,
                            ones_row,
                            bo_sb[:, n * 512:n * 512 + 512],
                            start=False, stop=True,
                        )
                        osb = opool.tile([128, 512], bf16, tag="osb")
                        nc.scalar.copy(osb, ops)
                        nc.sync.dma_start(
                            out=out[blk * 128:blk * 128 + 128, n * 512:n * 512 + 512],
                            in_=osb,
                        )
    return nc


def _host_prep(x, Wq, bq, Wk, bk, Wv, bv, Wo, bo):
    """Build per-core input maps."""
    # per-head even/odd deinterleave permutation of output columns
    def colperm(nheads):
        p = []
        for h in range(nheads):
            base = h * DK
            p.extend([base + 2 * j for j in range(HALF)])
            p.extend([base + 2 * j + 1 for j in range(HALF)])
        return np.array(p)

    qperm = colperm(N_HEAD)
    kperm = colperm(N_KV)
    wq_p = np.asarray(Wq, np.float32)[:, qperm]
    wk_p = np.asarray(Wk, np.float32)[:, kperm]
    bq_p = np.asarray(bq, np.float32)[qperm]
    bk_p = np.asarray(bk, np.float32)[kperm]
    wv_c = np.asarray(Wv, np.float32)
    bv_c = np.asarray(bv, np.float32)
    wo_c = np.asarray(Wo, np.float32)

    wq_pre = np.ascontiguousarray(
        wq_p.reshape(16, 128, 16, 128).transpose(1, 2, 0, 3)).astype(BF16)
    wo_pre = np.ascontiguousarray(
        wo_c.reshape(16, 128, 4, 512).transpose(1, 2, 0, 3)).astype(BF16)
    bq_pre = np.ascontiguousarray(bq_p.reshape(16, 128).T).astype(np.float32)
    bo_r = np.asarray(bo, np.float32).astype(BF16).reshape(1, D_MODEL)

    invf = THETA ** (-(np.arange(HALF, dtype=np.float64) * 2.0 / DK))
    posf = np.arange(S, dtype=np.float64)
    ang = posf[:, None] * invf[None, :]  # [S, 32]
    cos_t, sin_t = np.cos(ang), np.sin(ang)

    def rope_tables(pos_idx):
        n = len(pos_idx)
        C = np.zeros((128, n), np.float32)
        D = np.zeros((128, n), np.float32)
        for p in range(128):
            r = p % DK
            i = r if r < HALF else r - HALF
            C[p] = cos_t[pos_idx, i]
            D[p] = (-sin_t[pos_idx, i]) if r < HALF else sin_t[pos_idx, i]
        return C.astype(BF16), D.astype(BF16)

    ckt, dkt = rope_tables(np.arange(S))

    psw = np.zeros((128, 128), np.float32)
    for m in range(128):
        k = m + HALF if (m % DK) < HALF else m - HALF
        psw[k, m] = 1.0
    psw = psw.astype(BF16)
    pka = np.zeros((64, 128), np.float32)
    pkb = np.zeros((64, 128), np.float32)
    for k in range(64):
        pka[k, k] = 1.0
        pkb[k, k + 64] = 1.0
    pka, pkb = pka.astype(BF16), pkb.astype(BF16)
    sel2 = np.zeros((2, 128), np.float32)
    sel2[0, 0:64] = 1.0
    sel2[1, 64:128] = 1.0
    sel2 = sel2.astype(BF16)
    ident = np.eye(128, dtype=np.float32).astype(BF16)

    in_maps = []
    meta = []
    for c in range(8):
        b, r = c // 4, c % 4
        blocks = [r, 7 - r]
        qrows = np.concatenate([np.arange(bb * 128, bb * 128 + 128) for bb in blocks])
        xb = np.asarray(x[b], dtype=np.float32)
        xT_pre = np.ascontiguousarray(
            xb.reshape(S, 16, 128).transpose(2, 1, 0)).astype(BF16)
        xq_pre = np.ascontiguousarray(
            xb[qrows].reshape(QT, 16, 128).transpose(2, 1, 0)).astype(BF16)
        wk_sh = np.ascontiguousarray(
            wk_p[:, 128 * r:128 * r + 128].reshape(16, 128, 128).transpose(1, 0, 2)
        ).astype(BF16)
        wv_sh = np.ascontiguousarray(
            wv_c[:, 128 * r:128 * r + 128].reshape(16, 128, 128).transpose(1, 0, 2)
        ).astype(BF16)
        bk_sh = np.ascontiguousarray(bk_p[128 * r:128 * r + 128].reshape(128, 1))
        bv_sh = np.ascontiguousarray(bv_c[128 * r:128 * r + 128].reshape(128, 1))
        cqt, dqt = rope_tables(qrows)
        # mask[kt_local, i, blk*128 + ql] = 1 if (i*128+kt_local) <= qpos else 0
        mask = np.zeros((128, NT, QT), np.float32)
        kt_local = np.arange(128)
        for i in range(NT):
            ktg = i * 128 + kt_local
            for blki, bb in enumerate(blocks):
                qpos = bb * 128 + np.arange(128)
                mask[:, i, blki * 128:blki * 128 + 128] = (
                    ktg[:, None] <= qpos[None, :]
                )
        in_maps.append({
            "xT": xT_pre, "xq": xq_pre, "wq": wq_pre, "wk": wk_sh, "wv": wv_sh,
            "wo": wo_pre,
            "bq": bq_pre, "bk": bk_sh, "bv": bv_sh, "bo": bo_r,
            "ckt": ckt, "dkt": dkt, "cqt": cqt, "dqt": dqt,
            "pswap": psw, "packa": pka, "packb": pkb, "sel2": sel2,
            "ident": ident,
            "maskT": mask.astype(BF16),
        })
        meta.append((b, blocks))
    return in_maps, meta


def kernel(x, Wq, bq, Wk, bk, Wv, bv, Wo, bo):
    if "nc" not in _cache:
        nc0 = _build_nc()
        nc0.finalize()
        _cache["nc"] = nc0
    nc = _cache["nc"]
    in_maps, meta = _host_prep(x, Wq, bq, Wk, bk, Wv, bv, Wo, bo)
    res = run_bass_kernel_spmd(nc, in_maps, list(range(8)))
    full = np.zeros((B, S, D_MODEL), np.float32)
    for c in range(8):
        b, blocks = meta[c]
        o = np.asarray(res.results[c]["out"], dtype=np.float32)
        for blki, bb in enumerate(blocks):
            full[b, bb * 128:bb * 128 + 128] = o[blki * 128:(blki + 1) * 128]
    return full
